# revision 21
# baseline (speedup 1.0000x reference)
"""Causal single-head attention (B=4, T=4096, D_in=1024, D_out=64) on 8 trn2 cores.

Sharding: 2 cores per batch. Within a pair, core h in {0,1} owns the k/v
positions in 256-wide blocks of parity h (even/odd), and computes partial
unnormalized attention for ALL 4096 queries over its k half, plus the
softmax row-sums (via a ones-column appended to V). The host sums the two
partials and normalizes. Causality lands symmetrically on both parities, so
one SPMD program serves all 8 cores; per-core behavior differs only through
data:

  - xT (x[b] transposed to [D,T]) with each 512-column tile's two 256-blocks
    swapped for h=1, so "even permuted block" = own-parity block on every core
  - a per-core [128, 1024] fp16 mask tile: in permuted coordinates the
    diagonal k-tile pair's causal mask is the SAME for every q-slot (a
    triangle on the own-parity query half-blocks; all-pass (h=0) or
    all-masked (h=1) on the other-parity half-blocks), so one host-built
    tile multiplied onto exp(scores) by a single DVE tensor_tensor per slot
    implements causality.

The whole pipeline runs in fp16 with fp32 PSUM accumulation. Scores pack two
c=64 k-tiles into the PE array's row halves (tile_position (0,0)/(64,0));
K^T and Q^T are duplicated into partitions 64..127 (Q via doubled projection
weights, K via one SBUF->SBUF DMA per stripe).

Schedule: all HBM loads share ONE DMA queue (sync) in priority order --
wkv, the leading x quarter, wq2, the next quarter, cpk, then the rest of
stripe 0 and stripes 1-3 -- so the 16 shared DMA engines drain them
strictly in that order at their aggregate-bandwidth cap (~360GB/s). All
weight/const DRAM layouts are partition-major so each partition line is
one fat descriptor (descriptor generation is ~2.5ns/descriptor, serialized
per queue, and feeds the engines). A short fp32 junk-matmul burst holds
the PE p-state/clock-gate open until the first quarter lands. Attention
slots are processed 0,1,2,3,6,7,4,5 and interleaved pair-by-pair with the
next stripe's projection matmuls, so the ACT engine (exp is ~39us of
work, the co-bottleneck) streams continuously from the first stripe
onward instead of idling through the projection phase; slots 6,7 (the
biggest) overlap stripe 3's Q projection, and slots 4,5 drain the tail.
"""

import sys
import types

import numpy as np

B, T, D, E = 4, 4096, 1024, 64
NCORES = 8
P = 128
HB = 256  # parity half-block width
NQT = 8  # q-slots of 512
DC = D // P  # 8 d-chunks

_cache = {}


def _sl(start, size):
    return slice(start, start + size)


def _build_program():
    import concourse.mybir as mybir
    import concourse.tile as tile
    from concourse import bacc

    f32 = mybir.dt.float32
    fp16 = mybir.dt.float16
    Exp = mybir.ActivationFunctionType.Exp
    Alu = mybir.AluOpType

    nc = bacc.Bacc("TRN2", target_bir_lowering=False, debug=False, num_devices=NCORES)

    xT = nc.dram_tensor("xT", [D, T], fp16, kind="ExternalInput")
    wkv = nc.dram_tensor("wkv", [P, DC * 2 * E], fp16, kind="ExternalInput")
    wq2 = nc.dram_tensor("wq2", [P, DC * P], fp16, kind="ExternalInput")
    # cpk: packed constants, per-partition:
    #   [ident(64) | ones(16) | pad(16) | diag-mask(1024)]
    cpk = nc.dram_tensor("cpk", [P, 96 + 1024], fp16, kind="ExternalInput")
    out = nc.dram_tensor("out", [E + 1, T], f32, kind="ExternalOutput")

    with tile.TileContext(nc) as tc:
        with (
            tc.tile_pool(name="const", bufs=1) as cpool,
            tc.tile_pool(name="persist", bufs=1) as ppool,
            tc.tile_pool(name="kvps", bufs=2, space="PSUM") as kvps,
            tc.tile_pool(name="sps", bufs=2, space="PSUM") as sps,
            tc.tile_pool(name="ops", bufs=2, space="PSUM") as ops,
            tc.tile_pool(name="exp", bufs=4) as exppool,
        ):
            kT_sb = ppool.tile([P, T // 2], fp16, name="kT")  # rows 64+: dup
            qT_sb = ppool.tile([P, T], fp16, name="qT")  # rows 64+: dup
            vT_tmp = ppool.tile([P, T // 2], fp16, name="vTt")  # rows 64+ used
            V_sb = ppool.tile([P, 16, E + 1], fp16, name="V")
            out_sb = ppool.tile([E + 1, T], f32, name="outsb")

            # PE warm-up on an uninitialized tile: no DMA dependency, so it
            # runs during the runtime preamble / first-stripe DMA window and
            # holds the HAM clock-gate open for the real work.
            junk_in = ppool.tile([P, 512], f32, name="junkin")
            nc.gpsimd.memset(junk_in[:], 0.0)
            junk_fp = ppool.tile([P, 256], fp16, name="junkfp")
            nc.gpsimd.memset(junk_fp[:], 0.0)
            warm = ops.tile([E + 1, 512], f32, tag="po")
            for _ in range(3):
                # fp32 matmuls run 4 cycles/row: long PE busy per instr
                nc.tensor.matmul(
                    warm[0 : P // 2, :],
                    junk_in[:, 0:E],
                    junk_in[:],
                    start=True,
                    stop=True,
                )

            # ---- DMA issuance: weights first (first consumers), x stripes
            # spread over four DGE queues so descriptor generation runs in
            # parallel. All weight/const DRAM layouts are partition-major so
            # each partition line is one fat descriptor.

            # All x-stripe DMAs go on the ONE sync queue in priority order:
            # the DMA engines drain a queue's descriptors in order, so
            # stripe-0 halves complete first instead of time-sharing the
            # engine pool with stripes 1-3. Halves (1KB descriptors) feed
            # descriptors fast enough (~400GB/s) to keep the engines at
            # their ~330GB/s cap.
            # One sync-queue priority order for everything needed early: the
            # DMA engines drain a single queue's descriptors in order, so the
            # first projection's deps (wkv, then the leading x quarter) are
            # not delayed by later transfers. wq2/cpk slot in after the first
            # quarter; stripes 1-3 follow.
            xT_view = xT.ap().rearrange("(c p) t -> p c t", p=P)
            wkv_sb = cpool.tile([P, DC, 2 * E], fp16)
            nc.sync.dma_start(
                wkv_sb[:], wkv.ap().rearrange("p (c w) -> p c w", w=2 * E)
            )
            xq = [ppool.tile([P, DC, HB], fp16, name=f"xq{i}") for i in range(2)]
            nc.sync.dma_start(xq[0][:], xT_view[:, :, 0:HB])
            wq2_sb = cpool.tile([P, DC, P], fp16)
            nc.sync.dma_start(
                wq2_sb[:], wq2.ap().rearrange("p (c w) -> p c w", w=P)
            )
            nc.sync.dma_start(xq[1][:], xT_view[:, :, HB : 2 * HB])
            xh1 = ppool.tile([P, DC, 512], fp16, name="xh1")
            nc.sync.dma_start(xh1[:], xT_view[:, :, 512:1024])
            cpk_sb = cpool.tile([P, 96 + 1024], fp16)
            nc.sync.dma_start(cpk_sb[:], cpk.ap())
            xts = [None] * 4
            for s in range(1, 4):
                xts[s] = ppool.tile([P, DC, 1024], fp16, name=f"xt{s}")
                nc.sync.dma_start(
                    xts[s][:], xT_view[:, :, 1024 * s : 1024 * (s + 1)]
                )

            # V ones column from the const pack
            nc.vector.tensor_copy(V_sb[:, :, E], cpk_sb[:, 64:80])
            ident = cpk_sb[:, 0:64]
            dmask = cpk_sb[:, 96 : 96 + 1024]

            # ---- projection emission helpers ----
            def emit_vtrans(j):
                vt = sps.tile([P, E], fp16, tag="ps")
                nc.tensor.transpose(
                    vt[:], vT_tmp[E:P, _sl(P * j, P)], ident[E:P, :]
                )
                nc.vector.tensor_copy(V_sb[:, j, 0:E], vt[:])

            def emit_stripe0():
                # kv block 0 from the leading quarter
                kvh = kvps.tile([P, HB], f32, tag="proj")
                for dc in range(DC):
                    nc.tensor.matmul(
                        kvh[:],
                        wkv_sb[:, dc, :],
                        xq[0][:, dc, :],
                        start=(dc == 0),
                        stop=(dc == DC - 1),
                    )
                nc.vector.tensor_copy(kT_sb[0:E, 0:HB], kvh[0:E, :])
                nc.vector.tensor_copy(vT_tmp[E:P, 0:HB], kvh[E:P, :])
                nc.gpsimd.dma_start(kT_sb[E:P, 0:HB], kT_sb[0:E, 0:HB])
                # q half 0 from the two quarters (groups sequential: the
                # PE's accumulation state is per tile position, so two
                # interleaved open groups would corrupt each other)
                q = kvps.tile([P, 512], f32, tag="proj")
                for i in range(2):
                    for dc in range(DC):
                        nc.tensor.matmul(
                            q[:, _sl(HB * i, HB)],
                            wq2_sb[:, dc, :],
                            xq[i][:, dc, :],
                            start=(dc == 0),
                            stop=(dc == DC - 1),
                        )
                nc.vector.tensor_copy(qT_sb[:, 0:512], q[:])
                # kv block 1 + q half 1 from the second half tile
                kvh = kvps.tile([P, HB], f32, tag="proj")
                for dc in range(DC):
                    nc.tensor.matmul(
                        kvh[:],
                        wkv_sb[:, dc, :],
                        xh1[:, dc, 0:HB],
                        start=(dc == 0),
                        stop=(dc == DC - 1),
                    )
                nc.vector.tensor_copy(kT_sb[0:E, _sl(HB, HB)], kvh[0:E, :])
                nc.vector.tensor_copy(vT_tmp[E:P, _sl(HB, HB)], kvh[E:P, :])
                nc.gpsimd.dma_start(
                    kT_sb[E:P, _sl(HB, HB)], kT_sb[0:E, _sl(HB, HB)]
                )
                q = kvps.tile([P, 512], f32, tag="proj")
                for dc in range(DC):
                    nc.tensor.matmul(
                        q[:],
                        wq2_sb[:, dc, :],
                        xh1[:, dc, :],
                        start=(dc == 0),
                        stop=(dc == DC - 1),
                    )
                nc.vector.tensor_copy(qT_sb[:, _sl(512, 512)], q[:])
                for j in range(4):
                    emit_vtrans(j)

            def gen_proj(s, vt_early=False, q_reverse=False):
                """Generator of stripe-s (s>=1) projection units; each yield
                is ~one PE matmul's worth of work."""
                kv = kvps.tile([P, 512], f32, tag="proj")
                xts_v = xts[s][:, :, :].rearrange("p c (h q) -> p c h q", q=512)
                for dc in range(DC):
                    nc.tensor.matmul(
                        kv[:],
                        wkv_sb[:, dc, :],
                        xts_v[:, dc, :, 0:HB],
                        start=(dc == 0),
                        stop=(dc == DC - 1),
                    )
                    yield
                m = 2 * s
                nc.vector.tensor_copy(kT_sb[0:E, _sl(HB * m, 512)], kv[0:E, :])
                nc.vector.tensor_copy(vT_tmp[E:P, _sl(HB * m, 512)], kv[E:P, :])
                nc.gpsimd.dma_start(
                    kT_sb[E:P, _sl(512 * s, 512)],
                    kT_sb[0:E, _sl(512 * s, 512)],
                )
                yield
                if vt_early:
                    for j in range(4 * s, 4 * s + 4):
                        emit_vtrans(j)
                        yield
                for half in ((1, 0) if q_reverse else (0, 1)):
                    q = kvps.tile([P, 512], f32, tag="proj")
                    for dc in range(DC):
                        nc.tensor.matmul(
                            q[:],
                            wq2_sb[:, dc, :],
                            xts[s][:, dc, _sl(512 * half, 512)],
                            start=(dc == 0),
                            stop=(dc == DC - 1),
                        )
                        yield
                    qt_i = 2 * s + half
                    nc.vector.tensor_copy(qT_sb[:, _sl(512 * qt_i, 512)], q[:])
                    yield
                if not vt_early:
                    for j in range(4 * s, 4 * s + 4):
                        emit_vtrans(j)
                        yield

            # ---- attention machinery ----
            pendings = []  # (qt, oi, j0, nkb, exp_tile, po)

            def issue_attnv(pend):
                qt, oi, j0, nkb, ex, po_t = pend
                n_groups = nkb // 2
                for jj in range(2):
                    j2 = j0 + jj
                    nc.tensor.matmul(
                        po_t[:],
                        V_sb[:, j2, :],
                        ex[:, _sl(512 * jj, 512)],
                        start=(oi == 0 and jj == 0),
                        stop=(oi == n_groups - 1 and jj == 1),
                    )
                if oi == n_groups - 1:
                    nc.vector.tensor_copy(out_sb[:, _sl(512 * qt, 512)], po_t[:])
                    nc.scalar.dma_start(
                        out.ap()[:, _sl(512 * qt, 512)],
                        out_sb[:, _sl(512 * qt, 512)],
                    )

            def gen_slots(slots):
                """Generator over attention pairs of the given slots; each
                yield is one pair (2 scores MMs + exp + lagged attnV)."""
                for qt in slots:
                    nkb = 2 * qt + 2
                    po = ops.tile([E + 1, 512], f32, tag="po")
                    # diagonal (masked) pair first: its mask latency hides
                    # behind the remaining unmasked pairs
                    order = [nkb - 2] + list(range(0, nkb - 2, 2))
                    for oi, j0 in enumerate(order):
                        ps = sps.tile([P, 1024], f32, tag="ps")
                        # two c=64 score matmuls run concurrently in the PE's
                        # row halves (lhsT/rhs partition halves carry dups)
                        nc.tensor.matmul(
                            ps[:, 0:512],
                            kT_sb[0:E, _sl(P * j0, P)],
                            qT_sb[0:E, _sl(512 * qt, 512)],
                            start=True,
                            stop=True,
                            tile_position=(0, 0),
                        )
                        nc.tensor.matmul(
                            ps[:, 512:1024],
                            kT_sb[E:P, _sl(P * (j0 + 1), P)],
                            qT_sb[E:P, _sl(512 * qt, 512)],
                            start=True,
                            stop=True,
                            tile_position=(64, 0),
                        )
                        ex = exppool.tile([P, 1024], fp16)
                        nc.scalar.activation(ex[:], ps[:], Exp, scale=0.125)
                        if oi == 0:
                            # causal mask on the diagonal pair: one fp16
                            # multiply by the per-core precomputed mask tile
                            nc.vector.tensor_mul(ex[:], ex[:], dmask)
                        pendings.append((qt, oi, j0, nkb, ex, po))
                        if len(pendings) > 1:
                            issue_attnv(pendings.pop(0))
                        yield

            filler_tile = [None]

            def emit_filler():
                # tiny dependency-free fp16 matmul (n=64, ~80ns): keeps the
                # PE p-state up through ACT-paced stretches and hides the
                # semaphore wake latency of the next dependent matmul. One
                # persistent PSUM tile (allocated after projections are done
                # with the pool) avoids per-filler pool churn.
                if filler_tile[0] is None:
                    filler_tile[0] = kvps.tile([P, HB], f32, tag="proj", name="fill")
                nc.tensor.matmul(
                    filler_tile[0][0:64, 0:64],
                    junk_fp[:, 0:64],
                    junk_fp[:, 0:64],
                    start=True,
                    stop=True,
                )

            def interleave(pair_gen, unit_gen, units_per_pair=3, drain=True):
                """Emit pairs and proj units round-robin, spreading units
                between pairs (pairs lead so ACT is fed first). With
                drain=False, leftover units stay in unit_gen for the next
                segment to consume between ITS pairs, instead of running as
                a pair-starved (ACT-idle) block here."""
                pdone = udone = False
                while not pdone:
                    try:
                        next(pair_gen)
                    except StopIteration:
                        pdone = True
                    if pdone:
                        if not drain:
                            return
                        n = 10**9
                    else:
                        n = units_per_pair
                    for _ in range(n):
                        try:
                            next(unit_gen)
                        except StopIteration:
                            udone = True
                            break
                    if udone and not pdone:
                        # no units left: emit remaining pairs back-to-back
                        for _ in pair_gen:
                            pass
                        pdone = True

            # ---- schedule ----
            # Natural slot order with pair-granular unit spreading, so no
            # stretch of the kernel is pair-free while proj units remain (a
            # pair-starved stretch idles ACT, the co-bottleneck, and pushes
            # the ACT-paced tail out). Unit counts are chosen so that every
            # pair's producers (its stripe's qT copy, kT dup, V transposes)
            # are emitted BEFORE the pair: a consumer emitted ahead of its
            # producer gets no dependency edge and reads garbage.
            #   p-gen unit indices (vt_early): kv 1-8, copies+dup 9,
            #   vtrans 10-13, qh0 14-21, qh0-copy 22, qh1 23-30, qh1-copy 31
            from itertools import chain as _chain

            emit_stripe0()
            # S1: slots 0,1 (3 pairs) over stripe-1 proj; slots 2,3 need all
            # of p1, so drain it here (ACT is supply-limited anyway).
            interleave(gen_slots([0, 1]), gen_proj(1, vt_early=True),
                       units_per_pair=3)
            # S2: slots 2,3 (7 pairs, deps all in p1) over most of p2
            p2 = gen_proj(2, vt_early=True)
            interleave(gen_slots([2, 3]), p2, units_per_pair=3, drain=False)
            next(p2)  # u22 = qh2-0 copy: slot 4's qT, needed pre-pair
            # S2+: slots 4,5 (11 pairs) over the rest of p2 and most of p3
            # -- this closes the former ~8us ACT hole at the S2->S3
            # transition. p3 projects q half 1 (slot 7's queries) BEFORE
            # half 0, so slot 7 can still overlap p3's tail units and only
            # slot 6 runs fully ACT-paced at the end.
            p3 = gen_proj(3, vt_early=True, q_reverse=True)
            u23 = _chain(p2, p3)
            interleave(gen_slots([4, 5]), u23, units_per_pair=3, drain=False)
            # S3: slot 7 (8 pairs) over p3's last units, slot 6 (7 pairs,
            # ACT-paced) drains the tail
            interleave(gen_slots([7]), u23, units_per_pair=1)
            interleave(gen_slots([6]), u23, units_per_pair=1)
            for pend in pendings:
                issue_attnv(pend)
            pendings.clear()

    nc.compile()
    return nc


def _host_inputs():
    ident = np.zeros((P, 64), dtype=np.float16)
    for p in range(P):
        ident[p, p % 64] = 1.0
    pp = np.arange(P)[:, None]
    qq = np.arange(HB)[None, :]
    cpks = []
    for h in range(2):
        cpk = np.zeros((P, 96 + 1024), dtype=np.float16)
        cpk[:, 0:64] = ident
        cpk[:, 64:80] = 1.0
        # diagonal-pair causal mask, layout [j(2), half(2), q(256)]:
        # own-parity half = triangle, other-parity half = all h01
        m = cpk[:, 96:].reshape(P, 2, 2, HB)
        for j in range(2):
            m[:, j, 0, :] = (qq - P * j - pp >= 0).astype(np.float16)
            m[:, j, 1, :] = 1.0 if h == 0 else 0.0
        cpks.append(cpk)
    return cpks


def _ensure_axon_hooks_stub():
    """bass_utils imports antenv.axon_hooks when BASS_TRACE is set; that
    module is absent in this image, so provide a no-op registry."""
    try:
        import antenv.axon_hooks  # noqa: F401
    except ImportError:
        m = types.ModuleType("antenv.axon_hooks")
        m._h = [None]
        m.set_axon_ntff_profile_hook = lambda h: m._h.__setitem__(0, h)
        m.get_axon_ntff_profile_hook = lambda: m._h[0]
        sys.modules["antenv.axon_hooks"] = m


def kernel(x, Wq, Wk, Wv):
    _ensure_axon_hooks_stub()
    from concourse.bass_utils import run_bass_kernel_spmd

    if "nc" not in _cache:
        _cache["nc"] = _build_program()
    nc = _cache["nc"]

    x = np.asarray(x, dtype=np.float32)
    Wq = np.asarray(Wq, dtype=np.float32)
    Wk = np.asarray(Wk, dtype=np.float32)
    Wv = np.asarray(Wv, dtype=np.float32)

    # partition-major weight layouts: [P, DC*width] so each partition line
    # is one contiguous DMA descriptor
    wkv = np.ascontiguousarray(
        np.concatenate([Wk, Wv], axis=1)
        .reshape(DC, P, 2 * E)
        .transpose(1, 0, 2)
        .reshape(P, DC * 2 * E)
        .astype(np.float16)
    )
    wq2 = np.ascontiguousarray(
        np.concatenate([Wq, Wq], axis=1)
        .reshape(DC, P, P)
        .transpose(1, 0, 2)
        .reshape(P, DC * P)
        .astype(np.float16)
    )
    cpks = _host_inputs()

    xT_all = x.transpose(0, 2, 1).astype(np.float16)  # [B, D, T]
    in_maps = []
    for c in range(NCORES):
        b, h = c // 2, c % 2
        xT = xT_all[b]
        if h == 1:  # swap 256-pairs so own-parity block is at even positions
            xT = xT.reshape(D, 8, 2, HB)[:, :, ::-1, :].reshape(D, T)
        in_maps.append(
            {
                "xT": np.ascontiguousarray(xT),
                "wkv": wkv,
                "wq2": wq2,
                "cpk": cpks[h],
            }
        )

    res = run_bass_kernel_spmd(nc, in_maps, list(range(NCORES)))
    _cache["last_res"] = res

    outp = np.empty((B, T, E), dtype=np.float32)
    for b in range(B):
        U = np.zeros((E + 1, T), dtype=np.float64)
        for h in range(2):
            u = res.results[2 * b + h]["out"].astype(np.float64)
            if h == 1:
                u = u.reshape(E + 1, 8, 2, HB)[:, :, ::-1, :].reshape(E + 1, T)
            U += u
        outp[b] = (U[:E] / U[E : E + 1]).T.astype(np.float32)
    return outp
